# revision 31
# baseline (speedup 1.0000x reference)
"""Trainium2 Bass kernel for nn_Attention_45406394253435 (gnn segment attention).

Full-input contract: kernel(**inputs) takes the unsharded numpy inputs and
returns the full [N, C] output. Internally shards across 8 NeuronCores at
segment boundaries (batch is sorted), runs a Bass/Tile kernel per core, and
gathers.

Math (per point i in segment b):
    qp   = Wq q_i                               # device (big matmul, bf16)
    kbs  = (Wk k + bk)/sqrt(DH) ; vp = Wv v + bv  # host (tiny tables)
    e    = exp(kbs[b] * qp)                     # fused scalar-engine pass
    s[b] = sum_{i in b} e[i]  (ACT accum)       # minus pad-count correction
    out  = (e * vp[b]/s[b]) @ Wo.T + bo         # device (big matmul, bf16)
The max-subtraction in the reference softmax is omitted (attn is O(4), exp
safe in f32), and bq is dropped entirely: a per-(segment, channel) additive
constant in attn cancels exactly in the segment softmax.

Data layout: q and out live in DRAM as [128, 2*NP] bf16, partition-major —
for slot j, channel-block cb, column c of the slot:
    col = 2*offs[j] + cb*sp_j + c  holds  q[start_b + c, cb*128 + p]
so every per-slot DMA is 128 fully-contiguous runs (few descriptors).
"""

import math

import numpy as np

N = 131072
B = 64
C = 256
H = 8
DH = C // H
NCORES = 8
SEGS_PER_CORE = B // NCORES  # 8 slots per core
NB = C // 128  # channel partition blocks (2)


def _build_bass(slot_pads, repeats=1, timing_io=False):
    import contextlib

    import concourse.bacc as bacc
    import concourse.mybir as mybir
    import concourse.tile as tile

    f32 = mybir.dt.float32
    bf16 = mybir.dt.bfloat16

    slot_pads = tuple(slot_pads)
    NP = sum(slot_pads)
    offs = [0]
    for sp in slot_pads:
        offs.append(offs[-1] + sp)
    max_pad = max(slot_pads)

    # per-slot point groups (chunks of <=512, multiples of 64)
    def mk_groups(sp):
        gs, off = [], 0
        while off < sp:
            w = min(512, sp - off)
            gs.append((off, w))
            off += w
        return gs

    slot_groups = [mk_groups(sp) for sp in slot_pads]
    NGMAX = max(len(g) for g in slot_groups)
    assert NGMAX <= 8

    nc = bacc.Bacc("TRN2", target_bir_lowering=False, debug=False,
                   num_devices=NCORES)

    q_cols = 2 * (max_pad if timing_io else NP)
    qT_d = nc.dram_tensor("qT", [128, q_cols], bf16, kind="ExternalInput").ap()
    kbs_d = nc.dram_tensor("kbs", [128, NB * SEGS_PER_CORE], f32,
                           kind="ExternalInput").ap()
    vp_d = nc.dram_tensor("vp", [128, NB * SEGS_PER_CORE], f32,
                          kind="ExternalInput").ap()
    corr_d = nc.dram_tensor("corr", [128, NB * SEGS_PER_CORE], f32,
                            kind="ExternalInput").ap()
    wqt_d = nc.dram_tensor("wqt", [C, C], bf16, kind="ExternalInput").ap()
    wot_d = nc.dram_tensor("wot", [C, C], f32, kind="ExternalInput").ap()
    bo_d = nc.dram_tensor("bo_b", [128, NB], f32, kind="ExternalInput").ap()
    out_d = nc.dram_tensor("out", [128, q_cols], bf16,
                           kind="ExternalOutput").ap()

    with tile.TileContext(nc) as tc:
        with (
            tc.tile_pool(name="const", bufs=1) as cpool,
            tc.tile_pool(name="qp", bufs=3) as qpool,
            tc.tile_pool(name="ep", bufs=3) as epool,
            tc.tile_pool(name="sp", bufs=2) as spool,
            tc.tile_pool(name="wp", bufs=2) as wpool,
            tc.tile_pool(name="op", bufs=2) as opool,
            tc.tile_pool(name="ps1", bufs=1, space="PSUM") as ps1,
            tc.tile_pool(name="ps2", bufs=2, space="PSUM") as ps2,
        ):
            # constants — issued on the scalar HWDGE ring so the sync ring
            # starts streaming q immediately; wqt first (first matmul dep)
            wqt_t, wot_t = [], []
            for cb in range(NB):
                t = cpool.tile([128, C], bf16, tag=f"wqt{cb}")
                nc.scalar.dma_start(t[:], wqt_d[cb * 128:(cb + 1) * 128, :])
                wqt_t.append(t)
            kbs_t = cpool.tile([128, NB * SEGS_PER_CORE], f32, tag="kbs")
            nc.scalar.dma_start(kbs_t[:], kbs_d[:])
            for cb in range(NB):
                t = cpool.tile([128, C], f32, tag=f"wot{cb}")
                nc.scalar.dma_start(t[:], wot_d[cb * 128:(cb + 1) * 128, :])
                wot_t.append(t)
            vp_t = cpool.tile([128, NB * SEGS_PER_CORE], f32, tag="vp")
            nc.scalar.dma_start(vp_t[:], vp_d[:])
            corr_t = cpool.tile([128, NB * SEGS_PER_CORE], f32, tag="corr")
            nc.scalar.dma_start(corr_t[:], corr_d[:])
            bo_t = cpool.tile([128, NB], f32, tag="bo")
            nc.scalar.dma_start(bo_t[:], bo_d[:])

            rep_ctx = (tc.For_i(0, repeats, 1) if repeats > 1
                       else contextlib.nullcontext())
            with rep_ctx:
                _emit_body(nc, tc, mybir, slot_pads, offs, slot_groups,
                           NGMAX, qpool, epool, spool, wpool, opool, ps1,
                           ps2, qT_d, out_d, wqt_t, wot_t, kbs_t, vp_t,
                           corr_t, bo_t, timing_io)

    nc.compile()
    return nc


def _emit_body(nc, tc, mybir, slot_pads, offs, slot_groups, NGMAX,
               qpool, epool, spool, wpool, opool, ps1, ps2,
               qT_d, out_d, wqt_t, wot_t, kbs_t, vp_t, corr_t, bo_t,
               timing_io=False):
    f32 = mybir.dt.float32
    bf16 = mybir.dt.bfloat16
    Exp = mybir.ActivationFunctionType.Exp
    Ident = mybir.ActivationFunctionType.Identity
    X = mybir.AxisListType.X

    def phase1(j):
        sp = slot_pads[j]
        base = 2 * (0 if timing_io else offs[j])
        qm = qpool.tile([128, NB, sp], bf16, tag="q", name=f"q_{j}")
        npc = 4 if j == 0 else 1  # finer pieces on slot 0: earlier first MM
        for cb in range(NB):
            pw = -(-sp // npc) // 64 * 64
            for p0 in range(0, sp, pw):
                p1 = min(sp, p0 + pw)
                nc.sync.dma_start(
                    qm[:, cb, p0:p1],
                    qT_d[:, base + cb * sp + p0:base + cb * sp + p1])

        e_t = [epool.tile([128, sp], bf16, tag=f"e{cb}",
                          name=f"e{cb}_{j}") for cb in range(NB)]
        s_parts = spool.tile([128, NB * NGMAX], f32, tag="spart",
                             name=f"spart_{j}")

        # qp matmul + fused exp(scale*psum) + segment-sum accumulation.
        # Pairs of 512-col groups of the same cb share one [128,2,512]
        # PSUM tile so the exp runs as a single 1024-col instruction (the
        # scale is uniform within a cb), halving the ACT per-instruction
        # overhead (access latency + accumulator read).
        groups = slot_groups[j]
        ng = len(groups)
        nacc = [0, 0]
        for cb in range(NB):
            gp = 0
            while gp < ng:
                pair = groups[gp:gp + 2]
                pg = ps1.tile([128, 2, 512], f32, tag=f"p{cb}",
                              name=f"p{cb}_{j}_{gp}")
                for h, (off, w) in enumerate(pair):
                    for kb in range(NB):
                        nc.tensor.matmul(
                            pg[:, h, 0:w],
                            wqt_t[kb][:, cb * 128:(cb + 1) * 128],
                            qm[:, kb, off:off + w],
                            start=(kb == 0), stop=(kb == NB - 1))
                full = (len(pair) == 2 and pair[0][1] == 512
                        and pair[1][1] == 512)
                if full:
                    exps = [(e_t[cb][:, pair[0][0]:pair[0][0] + 1024],
                             pg[:, :, :])]
                else:
                    exps = [(e_t[cb][:, off:off + w], pg[:, h, 0:w])
                            for h, (off, w) in enumerate(pair)]
                for dst, src in exps:
                    a = cb * NGMAX + nacc[cb]
                    nc.scalar.activation(
                        dst, src, Exp,
                        scale=kbs_t[:, NB * j + cb:NB * j + cb + 1],
                        accum_out=s_parts[:, a:a + 1])
                    nacc[cb] += 1
                gp += 2
        assert nacc[0] == nacc[1] and nacc[0] <= NGMAX
        return e_t, s_parts, nacc[0]

    def phase2(j, e_t, s_parts, nacc):
        sp = slot_pads[j]
        groups = slot_groups[j]
        ng = len(groups)
        # finalize segment stats: s = accum - padcount, fold vp/s into Wo
        s_tot = spool.tile([128, NB], f32, tag="stot", name=f"stot_{j}")
        nc.vector.reduce_sum(
            s_tot[:],
            s_parts[:].rearrange("p (b g) -> p b g", b=NB)[:, :, 0:nacc],
            axis=X)
        s_val = spool.tile([128, NB], f32, tag="sval", name=f"sval_{j}")
        nc.vector.tensor_tensor(
            s_val[:], s_tot[:], corr_t[:, NB * j:NB * (j + 1)],
            op=mybir.AluOpType.subtract)
        r_t = spool.tile([128, NB], f32, tag="rt", name=f"rt_{j}")
        nc.vector.reciprocal(r_t[:], s_val[:])
        w_t = spool.tile([128, NB], f32, tag="wt", name=f"wt_{j}")
        nc.vector.tensor_tensor(
            w_t[:], vp_t[:, NB * j:NB * (j + 1)], r_t[:],
            op=mybir.AluOpType.mult)
        wp_t = []
        for cb in range(NB):
            wp = wpool.tile([128, C], bf16, tag=f"wp{cb}", name=f"wp{cb}_{j}")
            nc.vector.tensor_scalar_mul(wp[:], wot_t[cb][:], w_t[:, cb:cb + 1])
            wp_t.append(wp)

        # outT[c',pts] = (w*WoT)^T-stationary matmul over moving e, + bo.
        # Group pairs share one [128,2,512] PSUM tile so the bias-add/cast
        # runs as a single 1024-col instruction (amortizes ACT/DVE access
        # latency).
        out_stage = opool.tile([128, NB, sp], bf16, tag="ostage",
                               name=f"ostage_{j}")
        base = 2 * (0 if timing_io else offs[j])
        for cbp in range(NB):
            gp = 0
            while gp < ng:
                pair = groups[gp:gp + 2]
                po = ps2.tile([128, 2, 512], f32, tag="po",
                              name=f"po_{j}_{cbp}_{gp}")
                for h, (off, w) in enumerate(pair):
                    for kb in range(NB):
                        nc.tensor.matmul(
                            po[:, h, 0:w],
                            wp_t[kb][:, cbp * 128:(cbp + 1) * 128],
                            e_t[kb][:, off:off + w],
                            start=(kb == 0), stop=(kb == NB - 1))
                full = (len(pair) == 2 and pair[0][1] == 512
                        and pair[1][1] == 512)
                if full:
                    dst = out_stage[:, cbp, pair[0][0]:pair[0][0] + 1024]
                    srcs = [(dst, po[:, :, :])]
                else:
                    srcs = [(out_stage[:, cbp, off:off + w], po[:, h, 0:w])
                            for h, (off, w) in enumerate(pair)]
                for dst, src in srcs:
                    if outctr[0] % 5 == 0:
                        nc.scalar.activation(dst, src, Ident,
                                             bias=bo_t[:, cbp:cbp + 1])
                    else:
                        nc.vector.tensor_scalar_add(dst, src,
                                                    bo_t[:, cbp:cbp + 1])
                    outctr[0] += 1
                # store this chunk as soon as its bias-add lands (short tail)
                o0, w0 = pair[0][0], sum(w for _, w in pair)
                nc.gpsimd.dma_start(
                    out_d[:, base + cbp * sp + o0:base + cbp * sp + o0 + w0],
                    out_stage[:, cbp, o0:o0 + w0])
                gp += 2

    outctr = [0]
    prev = None
    for j in range(SEGS_PER_CORE):
        cur = phase1(j)
        if prev is not None:
            phase2(j - 1, *prev)
        prev = cur
    phase2(SEGS_PER_CORE - 1, *prev)


def _plan(batch):
    counts = np.bincount(np.asarray(batch).astype(np.int64), minlength=B)
    starts = np.concatenate([[0], np.cumsum(counts)])
    order = np.argsort(-counts, kind="stable")
    # rank-group r holds the r-th largest segment of each core; pads are the
    # per-rank-group maxima (shared across cores so the BASS build is SPMD)
    rank_pads = [
        max(256, int(-(-int(counts[order[SEGS_PER_CORE * r:
                                         SEGS_PER_CORE * (r + 1)]].max())
                       // 64) * 64))
        for r in range(SEGS_PER_CORE)]
    # slot order: smallest rank-group first (fast startup), second-smallest
    # last (short drain tail), big ones in the middle
    ranks = list(range(SEGS_PER_CORE))       # desc by size
    slot_rank = ([ranks[-1]] + ranks[:-2] + [ranks[-2]]
                 if SEGS_PER_CORE >= 2 else ranks)
    assign = [[int(order[SEGS_PER_CORE * r + c]) for r in slot_rank]
              for c in range(NCORES)]
    slot_pads = tuple(rank_pads[r] for r in slot_rank)
    offs = [0]
    for sp in slot_pads:
        offs.append(offs[-1] + sp)
    return counts, starts, assign, slot_pads, offs


def _host_prep(q, k, v, batch, Wq, bq, Wk, bk, Wv, bv, Wo, bo, plan):
    from ml_dtypes import bfloat16

    f = np.float32
    counts, starts, assign, slot_pads, offs = plan
    q = np.ascontiguousarray(q, dtype=f)
    kp = (np.asarray(k, f) @ np.asarray(Wk, f).T + np.asarray(bk, f))
    vp = (np.asarray(v, f) @ np.asarray(Wv, f).T + np.asarray(bv, f))
    kbs = kp / f(math.sqrt(DH))                     # [B, C]
    NP = offs[-1]

    in_maps = []
    wqt = np.ascontiguousarray(np.asarray(Wq, f).T.astype(bfloat16))
    wot = np.ascontiguousarray(np.asarray(Wo, f).T)
    bo_b = np.ascontiguousarray(np.asarray(bo, f).reshape(NB, 128).T)
    qbf = q.astype(bfloat16)
    for c in range(NCORES):
        qT = np.zeros((128, 2 * NP), dtype=bfloat16)
        kbs_c = np.empty((128, NB * SEGS_PER_CORE), dtype=f)
        vp_c = np.empty((128, NB * SEGS_PER_CORE), dtype=f)
        corr_c = np.empty((128, NB * SEGS_PER_CORE), dtype=f)
        for j in range(SEGS_PER_CORE):
            b = assign[c][j]
            n = counts[b]
            sp = slot_pads[j]
            blk = qbf[starts[b]:starts[b + 1]]
            for cb in range(NB):
                o = 2 * offs[j] + cb * sp
                qT[:, o:o + n] = blk[:, cb * 128:(cb + 1) * 128].T
                kbs_c[:, NB * j + cb] = kbs[b][cb * 128:(cb + 1) * 128]
                vp_c[:, NB * j + cb] = vp[b][cb * 128:(cb + 1) * 128]
                corr_c[:, NB * j + cb] = f(sp - n)
        in_maps.append({
            "qT": qT, "kbs": kbs_c, "vp": vp_c, "corr": corr_c,
            "wqt": wqt, "wot": wot, "bo_b": bo_b,
        })
    return in_maps


def _gather(results, plan):
    counts, starts, assign, slot_pads, offs = plan
    out = np.empty((N, C), dtype=np.float32)
    for c in range(NCORES):
        o = np.asarray(results[c]["out"]).astype(np.float32)
        for j in range(SEGS_PER_CORE):
            b = assign[c][j]
            n = counts[b]
            sp = slot_pads[j]
            for cb in range(NB):
                off = 2 * offs[j] + cb * sp
                out[starts[b]:starts[b + 1], cb * 128:(cb + 1) * 128] = \
                    o[:, off:off + n].T
    return out


_CACHE = {}


def _get_bass(slot_pads):
    if slot_pads not in _CACHE:
        _CACHE[slot_pads] = _build_bass(slot_pads)
    return _CACHE[slot_pads]


def kernel(q, k, v, batch, Wq, bq, Wk, bk, Wv, bv, Wo, bo):
    import concourse.bass_utils as bass_utils

    plan = _plan(batch)
    in_maps = _host_prep(q, k, v, batch, Wq, bq, Wk, bk, Wv, bv, Wo, bo, plan)
    nc = _get_bass(plan[3])

    last_err = None
    for attempt in range(3):  # device exec is rarely flaky; retry
        try:
            res = bass_utils.run_bass_kernel_spmd(
                nc, in_maps, core_ids=list(range(NCORES)))
            return _gather(res.results, plan)
        except Exception as e:  # noqa: BLE001
            last_err = e
            # Drop cached executables and give the device time to
            # self-recover before retrying in-process.
            import time

            try:
                import jax

                jax.clear_caches()
            except Exception:  # noqa: BLE001
                pass
            time.sleep(5 * (attempt + 1))
    raise last_err


# revision 34
# speedup vs baseline: 1.9680x; 1.9680x over previous
"""Trainium2 Bass kernel for nn_Attention_45406394253435 (gnn segment attention).

Full-input contract: kernel(**inputs) takes the unsharded numpy inputs and
returns the full [N, C] output. Internally shards across 8 NeuronCores at
segment boundaries (batch is sorted), runs a Bass/Tile kernel per core, and
gathers.

Math (per point i in segment b):
    qp   = Wq q_i                               # device (big matmul, bf16)
    kbs  = (Wk k + bk)/sqrt(DH) ; vp = Wv v + bv  # host (tiny tables)
    e    = exp(kbs[b] * qp)                     # fused scalar-engine pass
    s[b] = sum_{i in b} e[i]  (ACT accum)       # minus pad-count correction
    out  = (e * vp[b]/s[b]) @ Wo.T + bo         # device (big matmul, bf16)
The max-subtraction in the reference softmax is omitted (attn is O(4), exp
safe in f32), and bq is dropped entirely: a per-(segment, channel) additive
constant in attn cancels exactly in the segment softmax.

Data layout: q and out live in DRAM as [128, 2*NP] bf16, partition-major —
for slot j, channel-block cb, column c of the slot:
    col = 2*offs[j] + cb*sp_j + c  holds  q[start_b + c, cb*128 + p]
so every per-slot DMA is 128 fully-contiguous runs (few descriptors).
"""

import math

import numpy as np

N = 131072
B = 64
C = 256
H = 8
DH = C // H
NCORES = 8
SEGS_PER_CORE = B // NCORES  # 8 slots per core
NB = C // 128  # channel partition blocks (2)


def _build_bass(slot_pads, repeats=1, timing_io=False):
    import contextlib

    import concourse.bacc as bacc
    import concourse.mybir as mybir
    import concourse.tile as tile

    f32 = mybir.dt.float32
    bf16 = mybir.dt.bfloat16

    slot_pads = tuple(slot_pads)
    NP = sum(slot_pads)
    offs = [0]
    for sp in slot_pads:
        offs.append(offs[-1] + sp)
    max_pad = max(slot_pads)

    # per-slot point groups (chunks of <=512, multiples of 64)
    def mk_groups(sp):
        gs, off = [], 0
        while off < sp:
            w = min(512, sp - off)
            gs.append((off, w))
            off += w
        return gs

    slot_groups = [mk_groups(sp) for sp in slot_pads]
    NGMAX = max(len(g) for g in slot_groups)
    assert NGMAX <= 8

    nc = bacc.Bacc("TRN2", target_bir_lowering=False, debug=False,
                   num_devices=NCORES)

    # timing_io keeps the q upload small (every slot reads the same region —
    # reads may overlap) but the output stays full-size: overlapping HBM
    # writes from different slots would serialize on fake WAW conflicts.
    q_cols = 2 * (max_pad if timing_io else NP)
    qT_d = nc.dram_tensor("qT", [128, q_cols], bf16, kind="ExternalInput").ap()
    kbs_d = nc.dram_tensor("kbs", [128, NB * SEGS_PER_CORE], f32,
                           kind="ExternalInput").ap()
    vp_d = nc.dram_tensor("vp", [128, NB * SEGS_PER_CORE], f32,
                          kind="ExternalInput").ap()
    corr_d = nc.dram_tensor("corr", [128, NB * SEGS_PER_CORE], f32,
                            kind="ExternalInput").ap()
    wqt_d = nc.dram_tensor("wqt", [C, C], bf16, kind="ExternalInput").ap()
    wot_d = nc.dram_tensor("wot", [C, C], f32, kind="ExternalInput").ap()
    bo_d = nc.dram_tensor("bo_b", [128, NB], f32, kind="ExternalInput").ap()
    out_d = nc.dram_tensor("out", [128, 2 * NP], bf16,
                           kind="ExternalOutput").ap()

    with tile.TileContext(nc) as tc:
        with (
            tc.tile_pool(name="const", bufs=1) as cpool,
            tc.tile_pool(name="qp", bufs=3) as qpool,
            tc.tile_pool(name="ep", bufs=3) as epool,
            tc.tile_pool(name="sp", bufs=2) as spool,
            tc.tile_pool(name="wp", bufs=2) as wpool,
            tc.tile_pool(name="op", bufs=2) as opool,
            tc.tile_pool(name="ps1", bufs=1, space="PSUM") as ps1,
            tc.tile_pool(name="ps2", bufs=2, space="PSUM") as ps2,
        ):
            # constants — issued on the scalar HWDGE ring so the sync ring
            # starts streaming q immediately; wqt first (first matmul dep)
            wqt_t, wot_t = [], []
            for cb in range(NB):
                t = cpool.tile([128, C], bf16, tag=f"wqt{cb}")
                nc.scalar.dma_start(t[:], wqt_d[cb * 128:(cb + 1) * 128, :])
                wqt_t.append(t)
            kbs_t = cpool.tile([128, NB * SEGS_PER_CORE], f32, tag="kbs")
            nc.scalar.dma_start(kbs_t[:], kbs_d[:])
            for cb in range(NB):
                t = cpool.tile([128, C], f32, tag=f"wot{cb}")
                nc.scalar.dma_start(t[:], wot_d[cb * 128:(cb + 1) * 128, :])
                wot_t.append(t)
            vp_t = cpool.tile([128, NB * SEGS_PER_CORE], f32, tag="vp")
            nc.scalar.dma_start(vp_t[:], vp_d[:])
            corr_t = cpool.tile([128, NB * SEGS_PER_CORE], f32, tag="corr")
            nc.scalar.dma_start(corr_t[:], corr_d[:])
            bo_t = cpool.tile([128, NB], f32, tag="bo")
            nc.scalar.dma_start(bo_t[:], bo_d[:])

            rep_ctx = (tc.For_i(0, repeats, 1) if repeats > 1
                       else contextlib.nullcontext())
            with rep_ctx:
                _emit_body(nc, tc, mybir, slot_pads, offs, slot_groups,
                           NGMAX, qpool, epool, spool, wpool, opool, ps1,
                           ps2, qT_d, out_d, wqt_t, wot_t, kbs_t, vp_t,
                           corr_t, bo_t, timing_io)

    nc.compile()
    return nc


def _emit_body(nc, tc, mybir, slot_pads, offs, slot_groups, NGMAX,
               qpool, epool, spool, wpool, opool, ps1, ps2,
               qT_d, out_d, wqt_t, wot_t, kbs_t, vp_t, corr_t, bo_t,
               timing_io=False):
    f32 = mybir.dt.float32
    bf16 = mybir.dt.bfloat16
    Exp = mybir.ActivationFunctionType.Exp
    Ident = mybir.ActivationFunctionType.Identity
    X = mybir.AxisListType.X

    def phase1(j):
        sp = slot_pads[j]
        base = 2 * (0 if timing_io else offs[j])
        qm = qpool.tile([128, NB, sp], bf16, tag="q", name=f"q_{j}")
        npc = 4 if j == 0 else 1  # finer pieces on slot 0: earlier first MM
        for cb in range(NB):
            pw = -(-sp // npc) // 64 * 64
            for p0 in range(0, sp, pw):
                p1 = min(sp, p0 + pw)
                nc.sync.dma_start(
                    qm[:, cb, p0:p1],
                    qT_d[:, base + cb * sp + p0:base + cb * sp + p1])

        e_t = [epool.tile([128, sp], bf16, tag=f"e{cb}",
                          name=f"e{cb}_{j}") for cb in range(NB)]
        s_parts = spool.tile([128, NB * NGMAX], f32, tag="spart",
                             name=f"spart_{j}")

        # qp matmul + fused exp(scale*psum) + segment-sum accumulation.
        # Pairs of 512-col groups of the same cb share one [128,2,512]
        # PSUM tile so the exp runs as a single 1024-col instruction (the
        # scale is uniform within a cb), halving the ACT per-instruction
        # overhead (access latency + accumulator read).
        groups = slot_groups[j]
        ng = len(groups)
        nacc = [0, 0]
        for cb in range(NB):
            gp = 0
            while gp < ng:
                pair = groups[gp:gp + 2]
                pg = ps1.tile([128, 2, 512], f32, tag=f"p{cb}",
                              name=f"p{cb}_{j}_{gp}")
                for h, (off, w) in enumerate(pair):
                    for kb in range(NB):
                        nc.tensor.matmul(
                            pg[:, h, 0:w],
                            wqt_t[kb][:, cb * 128:(cb + 1) * 128],
                            qm[:, kb, off:off + w],
                            start=(kb == 0), stop=(kb == NB - 1))
                full = (len(pair) == 2 and pair[0][1] == 512
                        and pair[1][1] == 512)
                if full:
                    exps = [(e_t[cb][:, pair[0][0]:pair[0][0] + 1024],
                             pg[:, :, :])]
                else:
                    exps = [(e_t[cb][:, off:off + w], pg[:, h, 0:w])
                            for h, (off, w) in enumerate(pair)]
                for dst, src in exps:
                    a = cb * NGMAX + nacc[cb]
                    nc.scalar.activation(
                        dst, src, Exp,
                        scale=kbs_t[:, NB * j + cb:NB * j + cb + 1],
                        accum_out=s_parts[:, a:a + 1])
                    nacc[cb] += 1
                gp += 2
        assert nacc[0] == nacc[1] and nacc[0] <= NGMAX
        return e_t, s_parts, nacc[0]

    def phase2(j, e_t, s_parts, nacc):
        sp = slot_pads[j]
        groups = slot_groups[j]
        ng = len(groups)
        # finalize segment stats: s = accum - padcount, fold vp/s into Wo
        s_tot = spool.tile([128, NB], f32, tag="stot", name=f"stot_{j}")
        nc.vector.reduce_sum(
            s_tot[:],
            s_parts[:].rearrange("p (b g) -> p b g", b=NB)[:, :, 0:nacc],
            axis=X)
        s_val = spool.tile([128, NB], f32, tag="sval", name=f"sval_{j}")
        nc.vector.tensor_tensor(
            s_val[:], s_tot[:], corr_t[:, NB * j:NB * (j + 1)],
            op=mybir.AluOpType.subtract)
        r_t = spool.tile([128, NB], f32, tag="rt", name=f"rt_{j}")
        nc.vector.reciprocal(r_t[:], s_val[:])
        w_t = spool.tile([128, NB], f32, tag="wt", name=f"wt_{j}")
        nc.vector.tensor_tensor(
            w_t[:], vp_t[:, NB * j:NB * (j + 1)], r_t[:],
            op=mybir.AluOpType.mult)
        wp_t = []
        for cb in range(NB):
            wp = wpool.tile([128, C], bf16, tag=f"wp{cb}", name=f"wp{cb}_{j}")
            nc.vector.tensor_scalar_mul(wp[:], wot_t[cb][:], w_t[:, cb:cb + 1])
            wp_t.append(wp)

        # outT[c',pts] = (w*WoT)^T-stationary matmul over moving e, + bo.
        # Group pairs share one [128,2,512] PSUM tile so the bias-add/cast
        # runs as a single 1024-col instruction (amortizes ACT/DVE access
        # latency).
        out_stage = opool.tile([128, NB, sp], bf16, tag="ostage",
                               name=f"ostage_{j}")
        base = 2 * (0 if timing_io else offs[j])
        for cbp in range(NB):
            gp = 0
            while gp < ng:
                pair = groups[gp:gp + 2]
                po = ps2.tile([128, 2, 512], f32, tag="po",
                              name=f"po_{j}_{cbp}_{gp}")
                for h, (off, w) in enumerate(pair):
                    for kb in range(NB):
                        nc.tensor.matmul(
                            po[:, h, 0:w],
                            wp_t[kb][:, cbp * 128:(cbp + 1) * 128],
                            e_t[kb][:, off:off + w],
                            start=(kb == 0), stop=(kb == NB - 1))
                full = (len(pair) == 2 and pair[0][1] == 512
                        and pair[1][1] == 512)
                if full:
                    dst = out_stage[:, cbp, pair[0][0]:pair[0][0] + 1024]
                    srcs = [(dst, po[:, :, :])]
                else:
                    srcs = [(out_stage[:, cbp, off:off + w], po[:, h, 0:w])
                            for h, (off, w) in enumerate(pair)]
                for dst, src in srcs:
                    if outctr[0] % 5 == 0:
                        nc.scalar.activation(dst, src, Ident,
                                             bias=bo_t[:, cbp:cbp + 1])
                    else:
                        nc.vector.tensor_scalar_add(dst, src,
                                                    bo_t[:, cbp:cbp + 1])
                    outctr[0] += 1
                if j == SEGS_PER_CORE - 1:
                    # last slot: store per chunk so the drain tail is short
                    o0 = pair[0][0]
                    w0 = sum(w for _, w in pair)
                    nc.gpsimd.dma_start(
                        out_d[:, base + cbp * sp + o0:
                              base + cbp * sp + o0 + w0],
                        out_stage[:, cbp, o0:o0 + w0])
                gp += 2
            if j < SEGS_PER_CORE - 1:
                nc.gpsimd.dma_start(
                    out_d[:, base + cbp * sp:base + (cbp + 1) * sp],
                    out_stage[:, cbp, :])

    outctr = [0]
    prev = None
    for j in range(SEGS_PER_CORE):
        cur = phase1(j)
        if prev is not None:
            phase2(j - 1, *prev)
        prev = cur
    phase2(SEGS_PER_CORE - 1, *prev)


def _plan(batch):
    counts = np.bincount(np.asarray(batch).astype(np.int64), minlength=B)
    starts = np.concatenate([[0], np.cumsum(counts)])
    order = np.argsort(-counts, kind="stable")
    # rank-group r holds the r-th largest segment of each core; pads are the
    # per-rank-group maxima (shared across cores so the BASS build is SPMD)
    rank_pads = [
        max(256, int(-(-int(counts[order[SEGS_PER_CORE * r:
                                         SEGS_PER_CORE * (r + 1)]].max())
                       // 64) * 64))
        for r in range(SEGS_PER_CORE)]
    # slot order: smallest rank-group first (fast startup), second-smallest
    # last (short drain tail), big ones in the middle
    ranks = list(range(SEGS_PER_CORE))       # desc by size
    slot_rank = ([ranks[-1]] + ranks[:-2] + [ranks[-2]]
                 if SEGS_PER_CORE >= 2 else ranks)
    assign = [[int(order[SEGS_PER_CORE * r + c]) for r in slot_rank]
              for c in range(NCORES)]
    slot_pads = tuple(rank_pads[r] for r in slot_rank)
    offs = [0]
    for sp in slot_pads:
        offs.append(offs[-1] + sp)
    return counts, starts, assign, slot_pads, offs


def _host_prep(q, k, v, batch, Wq, bq, Wk, bk, Wv, bv, Wo, bo, plan):
    from ml_dtypes import bfloat16

    f = np.float32
    counts, starts, assign, slot_pads, offs = plan
    q = np.ascontiguousarray(q, dtype=f)
    kp = (np.asarray(k, f) @ np.asarray(Wk, f).T + np.asarray(bk, f))
    vp = (np.asarray(v, f) @ np.asarray(Wv, f).T + np.asarray(bv, f))
    kbs = kp / f(math.sqrt(DH))                     # [B, C]
    NP = offs[-1]

    in_maps = []
    wqt = np.ascontiguousarray(np.asarray(Wq, f).T.astype(bfloat16))
    wot = np.ascontiguousarray(np.asarray(Wo, f).T)
    bo_b = np.ascontiguousarray(np.asarray(bo, f).reshape(NB, 128).T)
    qbf = q.astype(bfloat16)
    for c in range(NCORES):
        qT = np.zeros((128, 2 * NP), dtype=bfloat16)
        kbs_c = np.empty((128, NB * SEGS_PER_CORE), dtype=f)
        vp_c = np.empty((128, NB * SEGS_PER_CORE), dtype=f)
        corr_c = np.empty((128, NB * SEGS_PER_CORE), dtype=f)
        for j in range(SEGS_PER_CORE):
            b = assign[c][j]
            n = counts[b]
            sp = slot_pads[j]
            blk = qbf[starts[b]:starts[b + 1]]
            for cb in range(NB):
                o = 2 * offs[j] + cb * sp
                qT[:, o:o + n] = blk[:, cb * 128:(cb + 1) * 128].T
                kbs_c[:, NB * j + cb] = kbs[b][cb * 128:(cb + 1) * 128]
                vp_c[:, NB * j + cb] = vp[b][cb * 128:(cb + 1) * 128]
                corr_c[:, NB * j + cb] = f(sp - n)
        in_maps.append({
            "qT": qT, "kbs": kbs_c, "vp": vp_c, "corr": corr_c,
            "wqt": wqt, "wot": wot, "bo_b": bo_b,
        })
    return in_maps


def _gather(results, plan):
    counts, starts, assign, slot_pads, offs = plan
    out = np.empty((N, C), dtype=np.float32)
    for c in range(NCORES):
        o = np.asarray(results[c]["out"]).astype(np.float32)
        for j in range(SEGS_PER_CORE):
            b = assign[c][j]
            n = counts[b]
            sp = slot_pads[j]
            for cb in range(NB):
                off = 2 * offs[j] + cb * sp
                out[starts[b]:starts[b + 1], cb * 128:(cb + 1) * 128] = \
                    o[:, off:off + n].T
    return out


_CACHE = {}


def _get_bass(slot_pads):
    if slot_pads not in _CACHE:
        _CACHE[slot_pads] = _build_bass(slot_pads)
    return _CACHE[slot_pads]


def kernel(q, k, v, batch, Wq, bq, Wk, bk, Wv, bv, Wo, bo):
    import concourse.bass_utils as bass_utils

    plan = _plan(batch)
    in_maps = _host_prep(q, k, v, batch, Wq, bq, Wk, bk, Wv, bv, Wo, bo, plan)
    nc = _get_bass(plan[3])

    last_err = None
    for attempt in range(3):  # device exec is rarely flaky; retry
        try:
            res = bass_utils.run_bass_kernel_spmd(
                nc, in_maps, core_ids=list(range(NCORES)))
            return _gather(res.results, plan)
        except Exception as e:  # noqa: BLE001
            last_err = e
            # Drop cached executables and give the device time to
            # self-recover before retrying in-process.
            import time

            try:
                import jax

                jax.clear_caches()
            except Exception:  # noqa: BLE001
                pass
            time.sleep(5 * (attempt + 1))
    raise last_err
